# revision 28
# baseline (speedup 1.0000x reference)
"""MoE ExpertLayer kernel for Trainium2 (8 NeuronCores, data-parallel over tokens).

Reference computation (B=4, S=2048, D=1024, E=8):
    logits  = x @ W_router.T + b_router          # [B,S,E]
    probs   = softmax(logits, axis=-1)
    y_e     = x @ W_experts[e].T + b_experts[e]  # all experts, dense
    out     = sum_e probs[..., e] * y_e          # [B,S,D]

Sharding: data-parallel over the flattened token axis (8192 tokens -> 1024
tokens per core). Every core receives the full (transposed) expert weights and
computes its token shard end-to-end; no collectives are needed.

Per-core dataflow (measured ~260us on HW, run-to-run spread ~260-266us from
device clock state; pure matmul-streaming floor for bf16 at 1 col/cycle/
2.4GHz is ~219us, so ~84% of roofline — the trace shows the remainder is
~7us NEFF preamble + ~2.5us first-DMA latency, a DMA-bound ramp (4MB of
xT + expert-0 weights over two ~125GB/s HWDGE rings gates full-rate expert
streaming until ~25us; the PE's HAM clock-gate also holds k=4/8 half rate
through the low-duty ramp), and a ~10us combine/store/drain tail; the
steady-state matmul stream runs gap-free at the 27.3us/expert floor):
  - All operands host-cast to bf16 and pre-tiled so the contraction dim d
    sits on SBUF partitions with contiguous 2KB-per-partition DMA rows.
    Rel err vs the fp32 reference ~3.0e-3 (scale-relative absmax).
  - xT [128, th, dt, t] resident in SBUF, both token halves in 2-dtile
    chunks (sync ring th0 / scalar ring th1) so each router half chases
    its chunks; expert 0's weights follow in 2-dtile chunks alternating
    across BOTH rings; experts 1..7 stream 1MB halves per ring.
  - Router: W_router.T stationary -> logits [8, 512] PSUM per token half
    (16 N=512 matmuls); the PSUM drain is a single fused ACT pass
    zT = exp(logits + b_router) (|logits| < ~4, so no max-subtraction
    needed), written in bf16. Per token tile, zT is transposed on the PE
    and normalized with three DVE ops (reduce_sum, reciprocal into a
    persistent recs tile, scalar mul) -> probs. This keeps the chain that
    gates expert 0's combines short: PE -> DVE x3, no ACT hop.
  - Bias fold: acc[t,f] = (sum_e zT[e,t]*b_e[f]) * rec[t], a K=8 matmul
    with zT stationary (no probs transpose needed). Emitted inside expert
    0's token loop so the matmuls fill expert 0's weight-chase gaps; the
    DVE rescale initializes acc right before the e=0 combine reads it.
  - Experts: per (token tile, f-half), PSUM accumulates 8 d-tile matmuls
    (the two f-halves share each stationary load); the combine
    acc = psum * probs[:,e] + acc is one fused DVE scalar_tensor_tensor op.
    The final expert's finished half-tiles stream straight out to DRAM.
Tried and rejected: fp8 e4m3 DoubleRow experts (2x PE rate) with a bf16
mean-expert pass and centered combine coefficients (p_e - 1/8) measures
2.6e-2 scale-relative absmax error — over the 2e-2 gate (tail tokens with
concentrated routing expose the full e4m3 quantization noise); K-split
bf16/fp8 hybrids that pass the gate keep only ~6% of the speedup. A PE
warm-up burst (dummy matmuls to ramp the HAM clock-gate early) made the
ramp worse: the gate re-throttles harder after the burst ends.
"""

import os
import sys

for _p in ("/opt/trn_rl_repo", "/root/.axon_site/_ro/trn_rl_repo"):
    if os.path.isdir(_p) and _p not in sys.path:
        sys.path.insert(0, _p)

from contextlib import ExitStack

import ml_dtypes
import numpy as np

import concourse.bass as bass
import concourse.mybir as mybir
import concourse.tile as tile
from concourse import bacc
from concourse.bass import ts
from concourse.bass_utils import run_bass_kernel_spmd
from concourse.masks import make_identity

B, S, D, E = 4, 2048, 1024, 8
N_CORES = 8
T = B * S // N_CORES  # tokens per core = 1024
P = 128               # partitions
TT = T // P           # token tiles per core = 8
DT = D // P           # contraction tiles = 8
FN = 512              # matmul moving free dim (one PSUM bank of fp32)
FH = D // FN          # output column halves = 2

MODE = os.environ.get("KERNEL_MODE", "bf16")  # bf16 | f32r | f32


def _compute_dt(mode):
    return {
        "bf16": mybir.dt.bfloat16,
        "f32r": mybir.dt.float32r,
        "f32": mybir.dt.float32,
    }[mode]


def _np_dt(mode):
    return {"bf16": ml_dtypes.bfloat16, "f32r": np.float32, "f32": np.float32}[mode]


def build(mode=MODE):
    """Build the per-core Bass/Tile program (identical SPMD program on all cores)."""
    cdt = _compute_dt(mode)
    f32 = mybir.dt.float32

    nc = bacc.Bacc("TRN2", target_bir_lowering=False, debug=False)

    # Inputs are pre-tiled on the host to [partition, ..., d-tile, ...] so
    # every DMA reads long contiguous per-partition chunks — the naive
    # [D, ...] layout yields 2KB strided descriptors that throttle a HWDGE
    # queue. xT is additionally split by token half so the router can start
    # as soon as the first half lands.
    TH = 2          # token halves per core
    THT = T // TH   # 512 tokens per half
    xT_d = nc.dram_tensor("xT", [P, TH, DT, THT], cdt, kind="ExternalInput").ap()
    Wt_d = nc.dram_tensor("Wt", [E, P, DT, D], cdt, kind="ExternalInput").ap()
    be_d = nc.dram_tensor("be", [E, D], cdt, kind="ExternalInput").ap()
    WrT_d = nc.dram_tensor("WrT", [P, DT, E], cdt, kind="ExternalInput").ap()
    brT_d = nc.dram_tensor("brT", [E, 1], f32, kind="ExternalInput").ap()
    out_d = nc.dram_tensor("out", [T, D], f32, kind="ExternalOutput").ap()

    with tile.TileContext(nc) as tc, ExitStack() as ctx:
        singles = ctx.enter_context(tc.tile_pool(name="singles", bufs=1))
        wpool = ctx.enter_context(tc.tile_pool(name="wpool", bufs=3))
        small = ctx.enter_context(tc.tile_pool(name="small", bufs=4))
        ppool = ctx.enter_context(tc.tile_pool(name="psum_e", bufs=2, space="PSUM"))
        pbias = ctx.enter_context(tc.tile_pool(name="psum_b", bufs=1, space="PSUM"))
        # pr double-buffered so router half 1's matmuls don't wait on half
        # 0's PSUM drain; the tiny z-transpose tile stays single-buffered.
        prout = ctx.enter_context(tc.tile_pool(name="psum_r", bufs=2, space="PSUM"))
        ptrans = ctx.enter_context(tc.tile_pool(name="psum_t", bufs=1, space="PSUM"))

        # Two HWDGE rings (sync=SP, scalar=ACT): spread big transfers across
        # both — a single ring saturates around ~120 GB/s for these patterns.
        hwdge = [nc.sync, nc.scalar]

        # Resident tensors. Tiny router tensors go first (they gate the router
        # phase), then the xT token-halves — one per ring — then weights.
        WrT = singles.tile([P, DT, E], cdt)
        nc.scalar.dma_start(out=WrT, in_=WrT_d)
        brT = singles.tile([E, 1], f32)
        nc.scalar.dma_start(out=brT, in_=brT_d)
        be = singles.tile([E, D], cdt)
        nc.sync.dma_start(out=be, in_=be_d)
        # xT th0 on sync / th1 on scalar, both in 2-dtile chunks so the two
        # router halves chase the chunks; expert 0's weights then split across
        # BOTH rings right behind the xT halves (see ramp below), so neither
        # ring idles during the router phase and expert 0 starts ~15us in.
        xT = singles.tile([P, TH, DT, THT], cdt)
        for c in range(0, DT, 2):
            nc.sync.dma_start(out=xT[:, 0, c : c + 2], in_=xT_d[:, 0, c : c + 2])
        for c in range(0, DT, 2):
            nc.scalar.dma_start(out=xT[:, 1, c : c + 2], in_=xT_d[:, 1, c : c + 2])
        identb = singles.tile([P, P], cdt)
        make_identity(nc, identb)

        acc = singles.tile([P, TT, D], f32)
        probs = singles.tile([P, TT, E], f32)
        # zT = exp(logits + b_router) in compute dtype: reused directly as the
        # bias-fold matmul stationary (no probs transpose needed); recs keeps
        # the per-token softmax reciprocal for the bias-fold rescale.
        zT = singles.tile([E, TT, P], cdt)
        recs = singles.tile([P, TT], f32)

        # ---- Router ----
        # logits[e, t] accumulate in PSUM with W_router as the stationary
        # (16 N=512 matmuls instead of 64 N=8 ones); the drain is one fused
        # ACT pass zT = exp(logits + b_router). Each token tile of zT is
        # transposed to [tok, e] on the PE and normalized with 3 DVE ops.
        out_dst = out_d.rearrange("(tt p) f -> p tt f", p=P)

        def router_half(th):
            t4 = slice(th * (TT // TH), (th + 1) * (TT // TH))
            pr = prout.tile([E, THT], f32, tag="pr")
            for dt_ in range(DT):
                nc.tensor.matmul(
                    pr, WrT[:, dt_, :], xT[:, th, dt_, :],
                    start=(dt_ == 0), stop=(dt_ == DT - 1),
                )
            # Fused drain: zT = exp(logits + b_router) in one ACT pass.
            # |logits| < ~4 here, so skipping the max-subtraction is safe in
            # fp32 and removes two engine hops from the per-tt chain that
            # gates expert 0's combines.
            nc.scalar.activation(
                out=zT[:, t4, :].rearrange("e a b -> e (a b)"), in_=pr,
                func=mybir.ActivationFunctionType.Exp, bias=brT, scale=1.0,
            )
            for tt in range(th * (TT // TH), (th + 1) * (TT // TH)):
                # token-major z, then probs = z / sum_e z with DVE ops only
                ptx = ptrans.tile([P, E], cdt, tag="ptx")
                nc.tensor.transpose(ptx, zT[:, tt, :], identb[:E, :E])
                ssum = small.tile([P, 1], f32, tag="ssum")
                nc.vector.reduce_sum(out=ssum, in_=ptx, axis=mybir.AxisListType.X)
                nc.vector.reciprocal(recs[:, tt : tt + 1], ssum)
                nc.vector.tensor_scalar_mul(
                    probs[:, tt, :], ptx, recs[:, tt : tt + 1]
                )

        def expert_block(e, w, tts):
            for tt in tts:
                # one stationary load serves both output halves: accumulate
                # the fh=0 and fh=1 PSUM groups side by side per d-tile
                pe0 = ppool.tile([P, FN], f32, tag="pe0")
                pe1 = ppool.tile([P, FN], f32, tag="pe1")
                for dt_ in range(DT):
                    lhsT = xT[:, tt // (TT // TH), dt_, ts(tt % (TT // TH), P)]
                    st = dt_ == 0
                    sp = dt_ == DT - 1
                    nc.tensor.matmul(
                        pe0, lhsT, w[:, dt_, 0:FN], start=st, stop=sp
                    )
                    nc.tensor.matmul(
                        pe1, lhsT, w[:, dt_, FN : 2 * FN], start=st, stop=sp
                    )
                if e == 0:
                    # bias fold acc[t,f] = (sum_e zT[e,t]*be[e,f]) * rec[t]
                    # = sum_e probs[t,e]*be[e,f]. Emitted here (not in the
                    # router) so these matmuls fill expert 0's w0 DMA-chase
                    # gaps; the DVE rescale initializes acc just before the
                    # e=0 combine below reads it.
                    for fh in range(FH):
                        pb = pbias.tile([P, FN], f32, tag="pb")
                        nc.tensor.matmul(
                            pb, zT[:, tt, :], be[:, ts(fh, FN)],
                            start=True, stop=True,
                        )
                        nc.vector.tensor_scalar_mul(
                            acc[:, tt, ts(fh, FN)], pb, recs[:, tt : tt + 1]
                        )
                # the very last tile combines/stores in quarter-columns so
                # its first bytes hit the ring sooner (trims the end-of-
                # kernel combine+store latency); everything else in halves
                last = e == E - 1 and tt == tts[-1]
                qn = FN // 2 if last else FN
                for fh, pe_ in ((0, pe0), (1, pe1)):
                    for q in range(FN // qn):
                        cs = fh * FN + q * qn
                        # acc = psum * probs[:, e] + acc  (one fused DVE op)
                        nc.vector.scalar_tensor_tensor(
                            out=acc[:, tt, cs : cs + qn],
                            in0=pe_[:, q * qn : (q + 1) * qn],
                            scalar=probs[:, tt, e : e + 1],
                            in1=acc[:, tt, cs : cs + qn],
                            op0=mybir.AluOpType.mult,
                            op1=mybir.AluOpType.add,
                        )
                        if e == E - 1:
                            # final expert: stream each finished chunk out now
                            # so stores overlap the remaining compute
                            hwdge[fh].dma_start(
                                out=out_dst[:, tt, cs : cs + qn],
                                in_=acc[:, tt, cs : cs + qn],
                            )

        # ---- Ramp: both router halves back-to-back (prout is double-
        # buffered so h1's PSUM group doesn't wait on h0's drain), while
        # expert 0's weights stream 2-dtile-chunk-wise across BOTH rings
        # right behind the xT halves. Expert 0 then runs all token tiles,
        # chasing the last w0 chunks as they land. ----
        half = DT // 2
        w0 = wpool.tile([P, DT, D], cdt, tag="w")
        # 2-dtile chunks alternating rings: d-tiles land in consumption order
        # (~2us cadence per ring), so expert 0's dt loop chases gap-free
        for i, c in enumerate(range(0, DT, 2)):
            hwdge[i % 2].dma_start(out=w0[:, c : c + 2], in_=Wt_d[0, :, c : c + 2])
        router_half(0)
        router_half(1)
        expert_block(0, w0, range(TT))

        # ---- Steady state: stream experts 1..7 across both HWDGE rings ----
        for e in range(1, E):
            w = wpool.tile([P, DT, D], cdt, tag="w")
            nc.sync.dma_start(out=w[:, :half, :], in_=Wt_d[e, :, :half, :])
            nc.scalar.dma_start(out=w[:, half:, :], in_=Wt_d[e, :, half:, :])
            expert_block(e, w, range(TT))

    nc.compile()
    return nc


def prep_inputs(x, W_experts, b_experts, W_router, b_router, mode=MODE):
    """Host-side marshalling: shard tokens, transpose so the contraction dim
    is DMA-contiguous onto SBUF partitions, cast to the compute dtype."""
    ndt = _np_dt(mode)
    x = np.asarray(x, dtype=np.float32).reshape(B * S, D)
    # [E, D_out, D_in] -> transposed + tiled to [E, P, DT, D_out] so each SBUF
    # partition reads one contiguous 16KB chunk per DMA
    Wt = np.ascontiguousarray(
        np.asarray(W_experts, dtype=np.float32)
        .transpose(0, 2, 1)            # [E, D_in, D_out]
        .reshape(E, DT, P, D)
        .transpose(0, 2, 1, 3)         # [E, P, DT, D_out]
    ).astype(ndt)
    WrT = np.ascontiguousarray(
        np.asarray(W_router, dtype=np.float32)
        .T.reshape(DT, P, E)
        .transpose(1, 0, 2)            # [P, DT, E]
    ).astype(ndt)
    be = np.asarray(b_experts, dtype=np.float32).astype(ndt)
    brT = np.asarray(b_router, dtype=np.float32).reshape(E, 1)
    TH, THT = 2, T // 2
    in_maps = []
    for c in range(N_CORES):
        xT = np.ascontiguousarray(
            x[c * T : (c + 1) * T, :]
            .T.reshape(DT, P, TH, THT)
            .transpose(1, 2, 0, 3)     # [P, TH, DT, THT]
        ).astype(ndt)
        in_maps.append({"xT": xT, "Wt": Wt, "be": be, "WrT": WrT, "brT": brT})
    return in_maps


_BUILT = {}


def get_built(mode=MODE):
    if mode not in _BUILT:
        _BUILT[mode] = build(mode)
    return _BUILT[mode]


def wait_device_ready(max_tries=8, sleep_s=20):
    """Poke the axon-tunneled devices until they respond. A crashed prior
    process can leave the remote exec unit wedged for a minute or two;
    the terminal recycles it on subsequent connection attempts."""
    import time

    import jax
    import jax.numpy as jnp

    for attempt in range(max_tries):
        try:
            devs = jax.devices()
            for d in devs[:1]:
                a = jax.device_put(jnp.ones((2, 2)), d)
                np.asarray(a)
            return True
        except Exception as exc:  # noqa: BLE001
            if attempt == max_tries - 1:
                raise
            print(f"device not ready (attempt {attempt + 1}): {exc}; retrying")
            time.sleep(sleep_s)
    return False


def run_spmd(in_maps, mode=MODE, **kwargs):
    nc = get_built(mode)
    wait_device_ready()
    try:
        return run_bass_kernel_spmd(
            nc, in_maps, core_ids=list(range(N_CORES)), **kwargs
        )
    except Exception as exc:  # noqa: BLE001
        print(f"run_bass_kernel_spmd failed ({exc}); retrying once after re-poke")
        wait_device_ready()
        return run_bass_kernel_spmd(
            nc, in_maps, core_ids=list(range(N_CORES)), **kwargs
        )


def kernel(x, W_experts, b_experts, W_router, b_router):
    in_maps = prep_inputs(x, W_experts, b_experts, W_router, b_router)
    res = run_spmd(in_maps)
    out = np.concatenate(
        [np.asarray(res.results[c]["out"], dtype=np.float32) for c in range(N_CORES)],
        axis=0,
    )
    return out.reshape(B, S, D)



# revision 30
# speedup vs baseline: 1.0279x; 1.0279x over previous
"""MoE ExpertLayer kernel for Trainium2 (8 NeuronCores, data-parallel over tokens).

Reference computation (B=4, S=2048, D=1024, E=8):
    logits  = x @ W_router.T + b_router          # [B,S,E]
    probs   = softmax(logits, axis=-1)
    y_e     = x @ W_experts[e].T + b_experts[e]  # all experts, dense
    out     = sum_e probs[..., e] * y_e          # [B,S,D]

Sharding: data-parallel over the flattened token axis (8192 tokens -> 1024
tokens per core). Every core receives the full (transposed) expert weights and
computes its token shard end-to-end; no collectives are needed.

Per-core dataflow (measured ~260us on HW, run-to-run spread ~260-266us from
device clock state; pure matmul-streaming floor for bf16 at 1 col/cycle/
2.4GHz is ~219us, so ~84% of roofline — the trace shows the remainder is
~7us NEFF preamble + ~2.5us first-DMA latency, a DMA-bound ramp (4MB of
xT + expert-0 weights over two ~125GB/s HWDGE rings gates full-rate expert
streaming until ~25us; the PE's HAM clock-gate also holds k=4/8 half rate
through the low-duty ramp), and a ~10us combine/store/drain tail; the
steady-state matmul stream runs gap-free at the 27.3us/expert floor):
  - All operands host-cast to bf16 and pre-tiled so the contraction dim d
    sits on SBUF partitions with contiguous 2KB-per-partition DMA rows.
    Rel err vs the fp32 reference ~3.0e-3 (scale-relative absmax).
  - xT [128, th, dt, t] resident in SBUF, both token halves in 2-dtile
    chunks (sync ring th0 / scalar ring th1) so each router half chases
    its chunks; expert 0's weights follow in 2-dtile chunks alternating
    across BOTH rings; experts 1..7 stream 1MB halves per ring.
  - Router: W_router.T stationary -> logits [8, 512] PSUM per token half
    (16 N=512 matmuls); the PSUM drain is a single fused ACT pass
    zT = exp(logits + b_router) (|logits| < ~4, so no max-subtraction
    needed), written in bf16. Per token tile, zT is transposed on the PE
    and normalized with three DVE ops (reduce_sum, reciprocal into a
    persistent recs tile, scalar mul) -> probs. This keeps the chain that
    gates expert 0's combines short: PE -> DVE x3, no ACT hop.
  - Bias fold: acc[t,f] = (sum_e zT[e,t]*b_e[f]) * rec[t], a K=8 matmul
    with zT stationary (no probs transpose needed). Emitted inside expert
    0's token loop so the matmuls fill expert 0's weight-chase gaps; the
    DVE rescale initializes acc right before the e=0 combine reads it.
  - Experts: per (token tile, f-half), PSUM accumulates 8 d-tile matmuls
    (the two f-halves share each stationary load); the combine
    acc = psum * probs[:,e] + acc is one fused DVE scalar_tensor_tensor op.
    The final expert's finished half-tiles stream straight out to DRAM.
Tried and rejected: fp8 e4m3 DoubleRow experts (2x PE rate) with a bf16
mean-expert pass and centered combine coefficients (p_e - 1/8) measures
2.6e-2 scale-relative absmax error — over the 2e-2 gate (tail tokens with
concentrated routing expose the full e4m3 quantization noise); K-split
bf16/fp8 hybrids that pass the gate keep only ~6% of the speedup. A PE
warm-up burst (dummy matmuls to ramp the HAM clock-gate early) made the
ramp worse: the gate re-throttles harder after the burst ends.
"""

import os
import sys

for _p in ("/opt/trn_rl_repo", "/root/.axon_site/_ro/trn_rl_repo"):
    if os.path.isdir(_p) and _p not in sys.path:
        sys.path.insert(0, _p)

from contextlib import ExitStack

import ml_dtypes
import numpy as np

import concourse.bass as bass
import concourse.mybir as mybir
import concourse.tile as tile
from concourse import bacc
from concourse.bass import ts
from concourse.bass_utils import run_bass_kernel_spmd
from concourse.masks import make_identity

B, S, D, E = 4, 2048, 1024, 8
N_CORES = 8
T = B * S // N_CORES  # tokens per core = 1024
P = 128               # partitions
TT = T // P           # token tiles per core = 8
DT = D // P           # contraction tiles = 8
FN = 512              # matmul moving free dim (one PSUM bank of fp32)
FH = D // FN          # output column halves = 2

MODE = os.environ.get("KERNEL_MODE", "bf16")  # bf16 | f32r | f32


def _compute_dt(mode):
    return {
        "bf16": mybir.dt.bfloat16,
        "f32r": mybir.dt.float32r,
        "f32": mybir.dt.float32,
    }[mode]


def _np_dt(mode):
    return {"bf16": ml_dtypes.bfloat16, "f32r": np.float32, "f32": np.float32}[mode]


def build(mode=MODE):
    """Build the per-core Bass/Tile program (identical SPMD program on all cores)."""
    cdt = _compute_dt(mode)
    f32 = mybir.dt.float32

    nc = bacc.Bacc("TRN2", target_bir_lowering=False, debug=False)

    # Inputs are pre-tiled on the host to [partition, ..., d-tile, ...] so
    # every DMA reads long contiguous per-partition chunks — the naive
    # [D, ...] layout yields 2KB strided descriptors that throttle a HWDGE
    # queue. xT is additionally split by token half so the router can start
    # as soon as the first half lands.
    TH = 2          # token halves per core
    THT = T // TH   # 512 tokens per half
    xT_d = nc.dram_tensor("xT", [P, TH, DT, THT], cdt, kind="ExternalInput").ap()
    Wt_d = nc.dram_tensor("Wt", [E, P, DT, D], cdt, kind="ExternalInput").ap()
    be_d = nc.dram_tensor("be", [E, D], cdt, kind="ExternalInput").ap()
    WrT_d = nc.dram_tensor("WrT", [P, DT, E], cdt, kind="ExternalInput").ap()
    brT_d = nc.dram_tensor("brT", [E, 1], f32, kind="ExternalInput").ap()
    out_d = nc.dram_tensor("out", [T, D], f32, kind="ExternalOutput").ap()

    with tile.TileContext(nc) as tc, ExitStack() as ctx:
        singles = ctx.enter_context(tc.tile_pool(name="singles", bufs=1))
        wpool = ctx.enter_context(tc.tile_pool(name="wpool", bufs=3))
        small = ctx.enter_context(tc.tile_pool(name="small", bufs=4))
        ppool = ctx.enter_context(tc.tile_pool(name="psum_e", bufs=2, space="PSUM"))
        pbias = ctx.enter_context(tc.tile_pool(name="psum_b", bufs=1, space="PSUM"))
        # pr double-buffered so router half 1's matmuls don't wait on half
        # 0's PSUM drain; the tiny z-transpose tile stays single-buffered.
        prout = ctx.enter_context(tc.tile_pool(name="psum_r", bufs=2, space="PSUM"))
        ptrans = ctx.enter_context(tc.tile_pool(name="psum_t", bufs=1, space="PSUM"))

        # Two HWDGE rings (sync=SP, scalar=ACT): spread big transfers across
        # both — a single ring saturates around ~120 GB/s for these patterns.
        hwdge = [nc.sync, nc.scalar]

        # Resident tensors. Tiny router tensors go first (they gate the router
        # phase), then the xT token-halves — one per ring — then weights.
        WrT = singles.tile([P, DT, E], cdt)
        nc.scalar.dma_start(out=WrT, in_=WrT_d)
        brT = singles.tile([E, 1], f32)
        nc.scalar.dma_start(out=brT, in_=brT_d)
        be = singles.tile([E, D], cdt)
        nc.sync.dma_start(out=be, in_=be_d)
        # xT th0 on sync / th1 on scalar, both in 2-dtile chunks so the two
        # router halves chase the chunks; expert 0's weights then split across
        # BOTH rings right behind the xT halves (see ramp below), so neither
        # ring idles during the router phase and expert 0 starts ~15us in.
        xT = singles.tile([P, TH, DT, THT], cdt)
        for c in range(0, DT, 2):
            nc.sync.dma_start(out=xT[:, 0, c : c + 2], in_=xT_d[:, 0, c : c + 2])
        for c in range(0, DT, 2):
            nc.scalar.dma_start(out=xT[:, 1, c : c + 2], in_=xT_d[:, 1, c : c + 2])
        identb = singles.tile([P, P], cdt)
        make_identity(nc, identb)

        acc = singles.tile([P, TT, D], f32)
        probs = singles.tile([P, TT, E], f32)
        # zT = exp(logits + b_router) in compute dtype: reused directly as the
        # bias-fold matmul stationary (no probs transpose needed); recs keeps
        # the per-token softmax reciprocal for the bias-fold rescale.
        zT = singles.tile([E, TT, P], cdt)
        recs = singles.tile([P, TT], f32)

        # ---- Router ----
        # logits[e, t] accumulate in PSUM with W_router as the stationary
        # (16 N=512 matmuls instead of 64 N=8 ones); the drain is one fused
        # ACT pass zT = exp(logits + b_router). Each token tile of zT is
        # transposed to [tok, e] on the PE and normalized with 3 DVE ops.
        out_dst = out_d.rearrange("(tt p) f -> p tt f", p=P)

        def router_half(th):
            t4 = slice(th * (TT // TH), (th + 1) * (TT // TH))
            pr = prout.tile([E, THT], f32, tag="pr")
            for dt_ in range(DT):
                nc.tensor.matmul(
                    pr, WrT[:, dt_, :], xT[:, th, dt_, :],
                    start=(dt_ == 0), stop=(dt_ == DT - 1),
                )
            # Fused drain: zT = exp(logits + b_router) in one ACT pass.
            # |logits| < ~4 here, so skipping the max-subtraction is safe in
            # fp32 and removes two engine hops from the per-tt chain that
            # gates expert 0's combines.
            nc.scalar.activation(
                out=zT[:, t4, :].rearrange("e a b -> e (a b)"), in_=pr,
                func=mybir.ActivationFunctionType.Exp, bias=brT, scale=1.0,
            )
            for tt in range(th * (TT // TH), (th + 1) * (TT // TH)):
                # token-major z, then probs = z / sum_e z with DVE ops only
                ptx = ptrans.tile([P, E], cdt, tag="ptx")
                nc.tensor.transpose(ptx, zT[:, tt, :], identb[:E, :E])
                ssum = small.tile([P, 1], f32, tag="ssum")
                nc.vector.reduce_sum(out=ssum, in_=ptx, axis=mybir.AxisListType.X)
                nc.vector.reciprocal(recs[:, tt : tt + 1], ssum)
                nc.vector.tensor_scalar_mul(
                    probs[:, tt, :], ptx, recs[:, tt : tt + 1]
                )

        def combine(e, tt, fh, pe_, qn):
            for q in range(FN // qn):
                cs = fh * FN + q * qn
                # acc = psum * probs[:, e] + acc  (one fused DVE op)
                nc.vector.scalar_tensor_tensor(
                    out=acc[:, tt, cs : cs + qn],
                    in0=pe_[:, q * qn : (q + 1) * qn],
                    scalar=probs[:, tt, e : e + 1],
                    in1=acc[:, tt, cs : cs + qn],
                    op0=mybir.AluOpType.mult,
                    op1=mybir.AluOpType.add,
                )
                if e == E - 1:
                    # final expert: stream each finished chunk out now
                    # so stores overlap the remaining compute
                    hwdge[fh].dma_start(
                        out=out_dst[:, tt, cs : cs + qn],
                        in_=acc[:, tt, cs : cs + qn],
                    )

        def expert0_half(fh, w):
            # Expert 0 runs one output half at a time across all token
            # tiles: its first matmul then needs only the fh=0 half of w0
            # (1MB, not 2MB), pulling the pipeline start earlier. The bias
            # fold rides along in the fh=0 pass, filling DMA-chase gaps.
            for tt in range(TT):
                pe_ = ppool.tile([P, FN], f32, tag=f"pe{fh}")
                for dt_ in range(DT):
                    lhsT = xT[:, tt // (TT // TH), dt_, ts(tt % (TT // TH), P)]
                    nc.tensor.matmul(
                        pe_, lhsT, w[:, dt_, ts(fh, FN)],
                        start=(dt_ == 0), stop=(dt_ == DT - 1),
                    )
                if fh == 0:
                    # bias fold acc[t,f] = (sum_e zT[e,t]*be[e,f]) * rec[t]
                    # = sum_e probs[t,e]*be[e,f], both halves of acc
                    for bfh in range(FH):
                        pb = pbias.tile([P, FN], f32, tag="pb")
                        nc.tensor.matmul(
                            pb, zT[:, tt, :], be[:, ts(bfh, FN)],
                            start=True, stop=True,
                        )
                        nc.vector.tensor_scalar_mul(
                            acc[:, tt, ts(bfh, FN)], pb, recs[:, tt : tt + 1]
                        )
                combine(0, tt, fh, pe_, FN)

        def expert_block(e, w, tts):
            for tt in tts:
                # one stationary load serves both output halves: accumulate
                # the fh=0 and fh=1 PSUM groups side by side per d-tile
                pe0 = ppool.tile([P, FN], f32, tag="pe0")
                pe1 = ppool.tile([P, FN], f32, tag="pe1")
                for dt_ in range(DT):
                    lhsT = xT[:, tt // (TT // TH), dt_, ts(tt % (TT // TH), P)]
                    st = dt_ == 0
                    sp = dt_ == DT - 1
                    nc.tensor.matmul(
                        pe0, lhsT, w[:, dt_, 0:FN], start=st, stop=sp
                    )
                    nc.tensor.matmul(
                        pe1, lhsT, w[:, dt_, FN : 2 * FN], start=st, stop=sp
                    )
                # the very last tile combines/stores in quarter-columns so
                # its first bytes hit the ring sooner (trims the end-of-
                # kernel combine+store latency); everything else in halves
                last = e == E - 1 and tt == tts[-1]
                qn = FN // 2 if last else FN
                for fh, pe_ in ((0, pe0), (1, pe1)):
                    combine(e, tt, fh, pe_, qn)

        # ---- Ramp: both router halves back-to-back (prout is double-
        # buffered so h1's PSUM group doesn't wait on h0's drain), while
        # expert 0's weights stream 2-dtile-chunk-wise across BOTH rings
        # right behind the xT halves. Expert 0 then runs all token tiles,
        # chasing the last w0 chunks as they land. ----
        half = DT // 2
        w0 = wpool.tile([P, DT, D], cdt, tag="w")
        # expert 0's fh=0 weight half streams on sync (behind xT th0) in
        # 2-dtile chunks, fh=1 on scalar (behind th1): the fh=0 pass can
        # start after only 1MB of w0 has landed, and the fh=1 half arrives
        # during the ~13.7us of fh=0 compute.
        for c in range(0, DT, 2):
            nc.sync.dma_start(
                out=w0[:, c : c + 2, 0:FN], in_=Wt_d[0, :, c : c + 2, 0:FN]
            )
        for c in range(0, DT, 2):
            nc.scalar.dma_start(
                out=w0[:, c : c + 2, FN:], in_=Wt_d[0, :, c : c + 2, FN:]
            )
        router_half(0)
        router_half(1)
        expert0_half(0, w0)
        expert0_half(1, w0)

        # ---- Steady state: stream experts 1..7 across both HWDGE rings ----
        for e in range(1, E):
            w = wpool.tile([P, DT, D], cdt, tag="w")
            nc.sync.dma_start(out=w[:, :half, :], in_=Wt_d[e, :, :half, :])
            nc.scalar.dma_start(out=w[:, half:, :], in_=Wt_d[e, :, half:, :])
            expert_block(e, w, range(TT))

    nc.compile()
    return nc


def prep_inputs(x, W_experts, b_experts, W_router, b_router, mode=MODE):
    """Host-side marshalling: shard tokens, transpose so the contraction dim
    is DMA-contiguous onto SBUF partitions, cast to the compute dtype."""
    ndt = _np_dt(mode)
    x = np.asarray(x, dtype=np.float32).reshape(B * S, D)
    # [E, D_out, D_in] -> transposed + tiled to [E, P, DT, D_out] so each SBUF
    # partition reads one contiguous 16KB chunk per DMA
    Wt = np.ascontiguousarray(
        np.asarray(W_experts, dtype=np.float32)
        .transpose(0, 2, 1)            # [E, D_in, D_out]
        .reshape(E, DT, P, D)
        .transpose(0, 2, 1, 3)         # [E, P, DT, D_out]
    ).astype(ndt)
    WrT = np.ascontiguousarray(
        np.asarray(W_router, dtype=np.float32)
        .T.reshape(DT, P, E)
        .transpose(1, 0, 2)            # [P, DT, E]
    ).astype(ndt)
    be = np.asarray(b_experts, dtype=np.float32).astype(ndt)
    brT = np.asarray(b_router, dtype=np.float32).reshape(E, 1)
    TH, THT = 2, T // 2
    in_maps = []
    for c in range(N_CORES):
        xT = np.ascontiguousarray(
            x[c * T : (c + 1) * T, :]
            .T.reshape(DT, P, TH, THT)
            .transpose(1, 2, 0, 3)     # [P, TH, DT, THT]
        ).astype(ndt)
        in_maps.append({"xT": xT, "Wt": Wt, "be": be, "WrT": WrT, "brT": brT})
    return in_maps


_BUILT = {}


def get_built(mode=MODE):
    if mode not in _BUILT:
        _BUILT[mode] = build(mode)
    return _BUILT[mode]


def wait_device_ready(max_tries=8, sleep_s=20):
    """Poke the axon-tunneled devices until they respond. A crashed prior
    process can leave the remote exec unit wedged for a minute or two;
    the terminal recycles it on subsequent connection attempts."""
    import time

    import jax
    import jax.numpy as jnp

    for attempt in range(max_tries):
        try:
            devs = jax.devices()
            for d in devs[:1]:
                a = jax.device_put(jnp.ones((2, 2)), d)
                np.asarray(a)
            return True
        except Exception as exc:  # noqa: BLE001
            if attempt == max_tries - 1:
                raise
            print(f"device not ready (attempt {attempt + 1}): {exc}; retrying")
            time.sleep(sleep_s)
    return False


def run_spmd(in_maps, mode=MODE, **kwargs):
    nc = get_built(mode)
    wait_device_ready()
    try:
        return run_bass_kernel_spmd(
            nc, in_maps, core_ids=list(range(N_CORES)), **kwargs
        )
    except Exception as exc:  # noqa: BLE001
        print(f"run_bass_kernel_spmd failed ({exc}); retrying once after re-poke")
        wait_device_ready()
        return run_bass_kernel_spmd(
            nc, in_maps, core_ids=list(range(N_CORES)), **kwargs
        )


def kernel(x, W_experts, b_experts, W_router, b_router):
    in_maps = prep_inputs(x, W_experts, b_experts, W_router, b_router)
    res = run_spmd(in_maps)
    out = np.concatenate(
        [np.asarray(res.results[c]["out"], dtype=np.float32) for c in range(N_CORES)],
        axis=0,
    )
    return out.reshape(B, S, D)

